# revision 9
# baseline (speedup 1.0000x reference)
"""Trainium2 Bass kernel for nn_DecoderModel_54795192762653.

4-layer decoder, B=4, T=1024, D=1024, H=16, K=4 kv-heads, HD=64, F=4096,
V=32000. 8 NeuronCores: pair (2b, 2b+1) handles batch b; within a pair,
core A owns tokens 0..511 and core B owns 512..1023.

Layout: activations are feature-major (channels on partitions, tokens on the
free axis). All big GEMMs run in fp32r (full-rate, ~1.5e-4 rel); the
attention-score matmuls run in bf16 (K=64 contraction, N=128 — fp32r would be
4x slower there).

Attention uses the reference's "scrambled" reshape semantics: unit m
(m = g*4 + kv) reads q rows m*64..(m+1)*64 (all channels) viewed as
(1024 l x 64 d); k/v block c = m % 4 rows c*256..(c+1)*256 viewed as
(1024 j x 64 d). Per core: units u*4+c for u in {0,1} (B's units are 8+same,
at identical local addresses). k/v for all tokens arrive via a pair
AllGather; the out-projection partial is pair Reduce-Scattered by token half.

Scores are computed transposed (j' on partitions, permuted j' = h4*256+tl),
masked with one affine_select per block, and the softmax denominator comes
free from a ones-column appended to V (M=65 matmul).
"""
import sys

sys.path.insert(0, "/opt/trn_rl_repo")

import numpy as np
from contextlib import ExitStack

import concourse.bass as bass
import concourse.tile as tile
from concourse import bacc, mybir
from concourse.bass_utils import run_bass_kernel_spmd
from concourse.masks import make_identity

P = 128
F32 = mybir.dt.float32
F32R = mybir.dt.float32r
BF16 = mybir.dt.bfloat16
U32 = mybir.dt.uint32
AF = mybir.ActivationFunctionType
OP = mybir.AluOpType

D, H, KV, F, L, V, T, B = 1024, 16, 4, 4096, 4, 32000, 1024, 4
HD = D // H
TL = T // 2          # 512 tokens per core
VC = V // 8          # 4000 vocab per core
EPS = 1e-5
PAIRS = [[0, 1], [2, 3], [4, 5], [6, 7]]
ALL8 = [list(range(8))]

_CACHE = {}


def _layer_norm(nc, pools, h_tiles, out_tiles, g_ap, b_ap):
    """Feature-major layernorm over D=1024 (8 partition tiles x 512 tokens)."""
    wk, ps_mm, ones_col = pools["wk"], pools["ps_mm"], pools["ones_col"]
    s1 = ps_mm.tile([P, 512], F32, name="mm")
    s2 = ps_mm.tile([P, 512], F32, name="mm")
    for r in range(8):
        nc.tensor.matmul(s1[0:1, :], ones_col[:, 0:1], h_tiles[r],
                         start=(r == 0), stop=(r == 7))
    for r in range(8):
        sq = wk.tile([P, 512], F32R, name="ln_sq")
        nc.scalar.activation(sq[:], h_tiles[r], AF.Square)
        nc.tensor.matmul(s2[0:1, :], ones_col[:, 0:1], sq[:],
                         start=(r == 0), stop=(r == 7))
    mu = wk.tile([1, 512], F32, name="ln_mu")
    nc.scalar.mul(mu[:], s1[0:1, :], 1.0 / D)
    e2 = wk.tile([1, 512], F32, name="ln_e2")
    nc.scalar.mul(e2[:], s2[0:1, :], 1.0 / D)
    musq = wk.tile([1, 512], F32, name="ln_musq")
    nc.scalar.activation(musq[:], mu[:], AF.Square)
    var = wk.tile([1, 512], F32, name="ln_var")
    nc.vector.tensor_sub(var[:], e2[:], musq[:])
    sd = wk.tile([1, 512], F32, name="ln_sd")
    nc.scalar.activation(sd[:], var[:], AF.Sqrt, bias=pools["eps"][0:1, :])
    rv = wk.tile([1, 512], F32, name="ln_rv")
    nc.vector.reciprocal(rv[:], sd[:])
    cv = wk.tile([1, 512], F32, name="ln_cv")
    nc.vector.scalar_tensor_tensor(cv[:], mu[:], -1.0, rv[:],
                                   op0=OP.mult, op1=OP.mult)
    rb = wk.tile([P, 512], F32, name="ln_rb")
    nc.gpsimd.partition_broadcast(rb[:], rv[:])
    cb = wk.tile([P, 512], F32, name="ln_cb")
    nc.gpsimd.partition_broadcast(cb[:], cv[:])
    for r in range(8):
        t1 = wk.tile([P, 512], F32, name="ln_t1")
        nc.vector.tensor_mul(t1[:], h_tiles[r], rb[:])
        nc.vector.tensor_add(t1[:], t1[:], cb[:])
        nc.scalar.activation(out_tiles[r], t1[:], AF.Identity,
                             bias=b_ap(r), scale=g_ap(r))


def build_kernel(n_layers=L):
    nc = bacc.Bacc("TRN2", target_bir_lowering=False, debug=False, num_devices=8)

    # ---------------- I/O ----------------
    idx_d = nc.dram_tensor("idx", [TL], U32, kind="ExternalInput")
    pe_d = nc.dram_tensor("pe", [TL, D], F32, kind="ExternalInput")
    emb_d = nc.dram_tensor("emb", [V, D], F32, kind="ExternalInput")
    wqkv_d = nc.dram_tensor("wqkv", [n_layers, D, 1536], F32R, kind="ExternalInput")
    wout_d = nc.dram_tensor("wout", [n_layers, TL, D], F32R, kind="ExternalInput")
    wup_d = nc.dram_tensor("wup", [n_layers, D, F], F32R, kind="ExternalInput")
    wgate_d = nc.dram_tensor("wgate", [n_layers, D, F], F32R, kind="ExternalInput")
    wdown_d = nc.dram_tensor("wdown", [n_layers, F, D], F32R, kind="ExternalInput")
    ln1g_d = nc.dram_tensor("ln1g", [n_layers, D], F32, kind="ExternalInput")
    ln1b_d = nc.dram_tensor("ln1b", [n_layers, D], F32, kind="ExternalInput")
    ln2g_d = nc.dram_tensor("ln2g", [n_layers, D], F32, kind="ExternalInput")
    ln2b_d = nc.dram_tensor("ln2b", [n_layers, D], F32, kind="ExternalInput")
    bup_d = nc.dram_tensor("bup", [n_layers, F], F32, kind="ExternalInput")
    bgate_d = nc.dram_tensor("bgate", [n_layers, F], F32, kind="ExternalInput")
    bdown_d = nc.dram_tensor("bdown", [n_layers, D], F32, kind="ExternalInput")
    flng_d = nc.dram_tensor("flng", [D], F32, kind="ExternalInput")
    flnb_d = nc.dram_tensor("flnb", [D], F32, kind="ExternalInput")
    wlm_d = nc.dram_tensor("wlm", [D, VC], F32R, kind="ExternalInput")
    blm_d = nc.dram_tensor("blm", [VC], F32, kind="ExternalInput")
    logits_d = nc.dram_tensor("logits", [B, VC], F32, kind="ExternalOutput")

    # collective bounce buffers (internal DRAM)
    kt_ag_in = nc.dram_tensor("kt_ag_in", [256, TL], BF16)
    kt_ag_out = nc.dram_tensor("kt_ag_out", [2, 256, TL], BF16)
    v_ag_in = nc.dram_tensor("v_ag_in", [TL, 256], F32)
    v_ag_out = nc.dram_tensor("v_ag_out", [2, TL, 256], F32)
    rs_in = nc.dram_tensor("rs_in", [2, 8, P, TL], F32)
    rs_out = nc.dram_tensor("rs_out", [8, P, TL], F32)
    fin_ag_in = nc.dram_tensor("fin_ag_in", [D], F32)
    fin_ag_out = nc.dram_tensor("fin_ag_out", [8, D], F32, addr_space="Shared")

    with tile.TileContext(nc) as tc, ExitStack() as ctx:
        pers = ctx.enter_context(tc.tile_pool(name="pers", bufs=1))
        wk = ctx.enter_context(tc.tile_pool(name="wk", bufs=2))
        ps_mm = ctx.enter_context(tc.tile_pool(name="ps_mm", bufs=3, space="PSUM"))
        ps_acc = ctx.enter_context(tc.tile_pool(name="ps_acc", bufs=1, space="PSUM"))
        pools = {"wk": wk, "ps_mm": ps_mm}

        # ---------------- constants ----------------
        ones_col = pers.tile([P, 1], F32R, tag="ones_col")
        nc.gpsimd.memset(ones_col[:].bitcast(F32), 1.0)
        pools["ones_col"] = ones_col
        ident = pers.tile([P, P], F32, tag="ident")
        make_identity(nc, ident[:])
        ones4 = pers.tile([1, 4], F32R, tag="ones4")
        nc.gpsimd.memset(ones4[:].bitcast(F32), 1.0)
        eps_t = pers.tile([P, 1], F32, tag="eps")
        nc.gpsimd.memset(eps_t[:], EPS)
        pools["eps"] = eps_t

        # ---------------- per-layer params (small, load all) ----------------
        lnp = {}
        for name, dram, nt in [("ln1g", ln1g_d, 8), ("ln1b", ln1b_d, 8),
                               ("ln2g", ln2g_d, 8), ("ln2b", ln2b_d, 8),
                               ("bup", bup_d, 32), ("bgate", bgate_d, 32),
                               ("bdown", bdown_d, 8)]:
            t = pers.tile([P, n_layers, nt], F32, tag=f"p_{name}")
            nc.sync.dma_start(t[:], dram.ap().rearrange("l (t p) -> p l t", p=P))
            lnp[name] = t
        fln = pers.tile([P, 2, 8], F32, tag="p_fln")
        nc.sync.dma_start(fln[:, 0], flng_d.ap().rearrange("(t p) -> p t", p=P))
        nc.sync.dma_start(fln[:, 1], flnb_d.ap().rearrange("(t p) -> p t", p=P))

        # ---------------- embedding ----------------
        h = pers.tile([P, 8, 512], F32R, tag="h")      # residual stream h^T
        with ExitStack() as ectx:
            ep = ectx.enter_context(tc.tile_pool(name="embed", bufs=2))
            idx_t = ep.tile([P, 4], U32, name="idx")
            nc.sync.dma_start(idx_t[:], idx_d.ap().rearrange("(c p) -> p c", p=P))
            for tt in range(4):
                g_nat = ep.tile([P, D], F32, name="g_nat")
                nc.gpsimd.indirect_dma_start(
                    out=g_nat[:], out_offset=None, in_=emb_d[:, :],
                    in_offset=bass.IndirectOffsetOnAxis(ap=idx_t[:, tt:tt + 1],
                                                        axis=0))
                pe_t = ep.tile([P, D], F32, name="pe_t")
                nc.sync.dma_start(pe_t[:], pe_d[tt * P:(tt + 1) * P, :])
                h_nat = ep.tile([P, D], F32, name="h_nat")
                nc.vector.scalar_tensor_tensor(h_nat[:], g_nat[:],
                                               float(np.sqrt(D)), pe_t[:],
                                               op0=OP.mult, op1=OP.add)
                for r in range(8):
                    tr_ps = ps_mm.tile([P, 512], F32, name="mm")
                    nc.tensor.transpose(tr_ps[:, 0:P],
                                        h_nat[:, r * P:(r + 1) * P], ident[:])
                    nc.scalar.copy(h[:, r, tt * P:(tt + 1) * P], tr_ps[:, 0:P])

        # ---------------- layers ----------------
        for ly in range(n_layers):
            with ExitStack() as lctx:
                ap_ = lctx.enter_context(tc.tile_pool(name=f"attn{ly}", bufs=1))
                apw = lctx.enter_context(tc.tile_pool(name=f"attnw{ly}", bufs=2))
                xh = ap_.tile([P, 8, 512], F32R, tag="xh")
                _layer_norm(nc, pools,
                            [h[:, r, :] for r in range(8)],
                            [xh[:, r, :] for r in range(8)],
                            lambda r: lnp["ln1g"][:, ly, r:r + 1],
                            lambda r: lnp["ln1b"][:, ly, r:r + 1])

                # ---- qkv ----
                qT = ap_.tile([P, 8, 512], BF16, tag="qT")
                kTl = ap_.tile([P, 2, 512], BF16, tag="kTl")
                for ct in range(10):
                    wc = apw.tile([P, 8, P], F32R, name="wqkv_ct")
                    nc.sync.dma_start(
                        wc[:], wqkv_d[ly, :, ct * P:(ct + 1) * P].rearrange(
                            "(kt p) c -> p kt c", p=P))
                    q_ps = ps_mm.tile([P, 512], F32, name="mm")
                    for kt in range(8):
                        nc.tensor.matmul(q_ps[:], wc[:, kt, :], xh[:, kt, :],
                                         start=(kt == 0), stop=(kt == 7))
                    if ct < 8:
                        nc.scalar.copy(qT[:, ct, :], q_ps[:])
                    else:
                        nc.scalar.copy(kTl[:, ct - 8, :], q_ps[:])
                wv = apw.tile([P, 8, 256], F32R, name="wv")
                nc.sync.dma_start(
                    wv[:], wqkv_d[ly, :, 1280:1536].rearrange(
                        "(kt p) c -> p kt c", p=P))
                vloc = ap_.tile([P, 4, 256], F32, tag="vloc")
                for tt in range(4):
                    v_ps = ps_mm.tile([P, 512], F32, name="mm")
                    for kt in range(8):
                        nc.tensor.matmul(v_ps[:, 0:256],
                                         xh[:, kt, tt * P:(tt + 1) * P],
                                         wv[:, kt, :],
                                         start=(kt == 0), stop=(kt == 7))
                    nc.vector.tensor_copy(vloc[:, tt, :], v_ps[:, 0:256])

                # ---- exchange k/v within pair ----
                nc.sync.dma_start(
                    kt_ag_in.ap().rearrange("(c p) t -> p c t", p=P), kTl[:])
                nc.gpsimd.collective_compute(
                    "AllGather", OP.bypass, replica_groups=PAIRS,
                    ins=[kt_ag_in[:, :]], outs=[kt_ag_out[:, :, :]])
                nc.sync.dma_start(
                    v_ag_in.ap().rearrange("(tt p) c -> p tt c", p=P), vloc[:])
                nc.gpsimd.collective_compute(
                    "AllGather", OP.bypass, replica_groups=PAIRS,
                    ins=[v_ag_in[:, :]], outs=[v_ag_out[:, :, :]])
                # kT duplicated on both partition halves: [128, 4 h4, 1024 t]
                kTd = ap_.tile([P, 4, T], BF16, tag="kTd")
                for half in range(2):
                    src = kt_ag_out[half].rearrange("(h4 d) t -> d h4 t", d=64)
                    nc.sync.dma_start(kTd[0:64, :, half * TL:(half + 1) * TL], src)
                    nc.sync.dma_start(kTd[64:128, :, half * TL:(half + 1) * TL], src)
                vst = ap_.tile([P, 8, 4, 65], F32R, tag="vst")
                nc.gpsimd.memset(vst[:, :, :, 64:65].bitcast(F32), 1.0)
                for h4 in range(4):
                    nc.gpsimd.dma_start(
                        vst[:, :, h4, 0:64],
                        v_ag_out.ap().rearrange(
                            "hf (tt p) c -> p (hf tt) c", p=P)
                        [:, :, h4 * 64:(h4 + 1) * 64])

                # ---- attention: 4 kv blocks x 2 units ----
                ost = [ap_.tile([P, 1024], F32R, tag=f"ost{r}", name=f"ost{r}") for r in range(4)]
                for c in range(4):
                    o_ps = [[ps_acc.tile([P, 512], F32, name=f"acc{u * 2 + lh}")
                             for lh in range(2)] for u in range(2)]
                    for jt in range(8):
                        h4, tlt = jt // 2, jt % 2
                        tl0 = tlt * P
                        a_chunk = apw.tile([P, 2, 8, 2, 64], F32R, name="a_chunk")
                        for par in range(2):
                            b0 = par * 64
                            for hq in range(2):
                                s_ps = ps_mm.tile([P, 4, 2, 64], F32, name="mm")
                                for hh in range(4):
                                    hidx = hq * 4 + hh
                                    lhsT = kTd[b0:b0 + 64, h4,
                                               c * 256 + tl0: c * 256 + tl0 + P]
                                    rhs = qT[b0:b0 + 64, hidx, :].rearrange(
                                        "p (blk tau) -> p blk tau",
                                        tau=64)[:, c::4, :]
                                    nc.tensor.matmul(s_ps[:, hh], lhsT, rhs,
                                                     start=True, stop=True)
                                nc.scalar.activation(
                                    a_chunk[:, par, hq * 4:(hq + 1) * 4], s_ps[:],
                                    AF.Exp, scale=0.125)
                        # mask: keep iff 16*tau + (2*hidx+par) - 4*(tl0+p) - h4 >= 0
                        nc.gpsimd.affine_select(
                            out=a_chunk[:], in_=a_chunk[:],
                            pattern=[[1, 2], [2, 8], [0, 2], [16, 64]],
                            channel_multiplier=-4, base=-(4 * tl0) - h4,
                            compare_op=OP.is_ge, fill=0.0)
                        tt8 = (c * 256 + tl0) // P
                        for u in range(2):
                            for lh in range(2):
                                rhs = a_chunk[:, :, :, u, lh * 32:(lh + 1) * 32]
                                nc.tensor.matmul(o_ps[u][lh][0:65, :],
                                                 vst[:, tt8, h4, :], rhs,
                                                 start=(jt == 0), stop=(jt == 7))
                    for u in range(2):
                        r = u * 2 + (c // 2)
                        for lh in range(2):
                            rcp = wk.tile([1, 512], F32, name="rcp")
                            nc.vector.reciprocal(rcp[:], o_ps[u][lh][64:65, :])
                            rcb = wk.tile([64, 512], F32, name="rcb")
                            nc.gpsimd.partition_broadcast(rcb[:], rcp[:])
                            nc.vector.tensor_mul(
                                ost[r][(c % 2) * 64:(c % 2) * 64 + 64,
                                       lh * 512:(lh + 1) * 512],
                                o_ps[u][lh][0:64, :], rcb[:])

                # ---- out-projection + pair reduce-scatter ----
                for rout in range(8):
                    woc = apw.tile([P, 4, P], F32R, name="wocol")
                    nc.sync.dma_start(
                        woc[:], wout_d[ly, :, rout * P:(rout + 1) * P].rearrange(
                            "(kt p) c -> p kt c", p=P))
                    for lh in range(2):
                        p_ps = ps_mm.tile([P, 512], F32, name="mm")
                        for kt in range(4):
                            rhs = ost[kt][:, lh * 512:(lh + 1) * 512].rearrange(
                                "p (par hidx tau) -> p tau hidx par",
                                par=2, hidx=8)
                            nc.tensor.matmul(p_ps[:], woc[:, kt, :], rhs,
                                             start=(kt == 0), stop=(kt == 3))
                        ap_sb = wk.tile([P, 512], F32, name="ap_sb")
                        nc.scalar.copy(ap_sb[:], p_ps[:])
                        nc.sync.dma_start(rs_in[lh, rout, :, :], ap_sb[:])
                nc.gpsimd.collective_compute(
                    "ReduceScatter", OP.add, replica_groups=PAIRS,
                    ins=[rs_in[:, :, :, :]], outs=[rs_out[:, :, :]])
                for r in range(8):
                    at = wk.tile([P, 512], F32, name="ap_sb")
                    nc.sync.dma_start(at[:], rs_out[r, :, :])
                    nc.vector.tensor_add(h[:, r, :], h[:, r, :], at[:])

            # ---------------- FFN ----------------
            with ExitStack() as fctx:
                fp = fctx.enter_context(tc.tile_pool(name=f"ffn{ly}", bufs=1))
                fpw = fctx.enter_context(tc.tile_pool(name=f"ffnw{ly}", bufs=2))
                fps = fctx.enter_context(tc.tile_pool(name=f"ffns{ly}", bufs=2))
                x2 = fp.tile([P, 8, 512], F32R, tag="x2")
                _layer_norm(nc, pools,
                            [h[:, r, :] for r in range(8)],
                            [x2[:, r, :] for r in range(8)],
                            lambda r: lnp["ln2g"][:, ly, r:r + 1],
                            lambda r: lnp["ln2b"][:, ly, r:r + 1])
                hg = fp.tile([P, 32, 512], F32R, tag="hg")
                for ch in range(16):          # F chunks of 256
                    wu = fpw.tile([P, 8, 256], F32R, name="wup")
                    nc.sync.dma_start(
                        wu[:], wup_d[ly, :, ch * 256:(ch + 1) * 256].rearrange(
                            "(kt p) c -> p kt c", p=P))
                    wg = fpw.tile([P, 8, 256], F32R, name="wgate")
                    nc.sync.dma_start(
                        wg[:], wgate_d[ly, :, ch * 256:(ch + 1) * 256].rearrange(
                            "(kt p) c -> p kt c", p=P))
                    for fi in range(2):       # F-tiles of 128 in chunk
                        ft = ch * 2 + fi
                        u_ps = ps_mm.tile([P, 512], F32, name="mm")
                        for kt in range(8):
                            nc.tensor.matmul(u_ps[:], wu[:, kt, fi * P:(fi + 1) * P],
                                             x2[:, kt, :],
                                             start=(kt == 0), stop=(kt == 7))
                        g_ps = ps_mm.tile([P, 512], F32, name="mm")
                        for kt in range(8):
                            nc.tensor.matmul(g_ps[:], wg[:, kt, fi * P:(fi + 1) * P],
                                             x2[:, kt, :],
                                             start=(kt == 0), stop=(kt == 7))
                        u_sb = fps.tile([P, 512], F32R, name="u_sb")
                        nc.scalar.activation(u_sb[:], u_ps[:], AF.Identity,
                                             bias=lnp["bup"][:, ly, ft:ft + 1])
                        g_sb = fps.tile([P, 512], F32R, name="g_sb")
                        nc.scalar.activation(g_sb[:], g_ps[:], AF.Gelu_apprx_tanh,
                                             bias=lnp["bgate"][:, ly, ft:ft + 1])
                        nc.vector.tensor_mul(hg[:, ft, :], u_sb[:], g_sb[:])
                # down: 2 groups of 4 out-tiles, Wdown streamed per group
                for grp in range(2):
                    d_ps = [ps_acc.tile([P, 512], F32, name=f"acc{i}")
                            for i in range(4)]
                    for kt in range(32):
                        wd = fpw.tile([P, 1024], F32R, name="wdown")
                        nc.sync.dma_start(wd[:],
                                          wdown_d[ly, kt * P:(kt + 1) * P, :])
                        for i in range(4):
                            rout = grp * 4 + i
                            nc.tensor.matmul(d_ps[i][:],
                                             wd[:, rout * P:(rout + 1) * P],
                                             hg[:, kt, :],
                                             start=(kt == 0), stop=(kt == 31))
                    for i in range(4):
                        rout = grp * 4 + i
                        dn = fps.tile([P, 512], F32, name="u_sb")
                        nc.scalar.activation(dn[:], d_ps[i][:], AF.Identity,
                                             bias=lnp["bdown"][:, ly, rout:rout + 1])
                        nc.vector.tensor_add(h[:, rout, :], h[:, rout, :], dn[:])

        # -------- final LN (local last token, col 511) + AG + LM head --------
        with ExitStack() as tctx:
            tp = tctx.enter_context(tc.tile_pool(name="tail", bufs=2))
            s1 = ps_mm.tile([P, 512], F32, name="mm")
            s2 = ps_mm.tile([P, 512], F32, name="mm")
            # fp32r matmuls need an even moving free dim: do 2 cols, use col 1
            for r in range(8):
                nc.tensor.matmul(s1[0:1, 0:2], ones_col[:, 0:1], h[:, r, 510:512],
                                 start=(r == 0), stop=(r == 7))
            for r in range(8):
                sqf = tp.tile([P, 2], F32R, name="fln_sq")
                nc.scalar.activation(sqf[:], h[:, r, 510:512], AF.Square)
                nc.tensor.matmul(s2[0:1, 0:2], ones_col[:, 0:1], sqf[:],
                                 start=(r == 0), stop=(r == 7))
            muf = tp.tile([1, 1], F32, name="fln_mu")
            nc.scalar.mul(muf[:], s1[0:1, 1:2], 1.0 / D)
            e2f = tp.tile([1, 1], F32, name="fln_e2")
            nc.scalar.mul(e2f[:], s2[0:1, 1:2], 1.0 / D)
            musqf = tp.tile([1, 1], F32, name="fln_musq")
            nc.scalar.activation(musqf[:], muf[:], AF.Square)
            varf = tp.tile([1, 1], F32, name="fln_var")
            nc.vector.tensor_sub(varf[:], e2f[:], musqf[:])
            sdf = tp.tile([1, 1], F32, name="fln_sd")
            nc.scalar.activation(sdf[:], varf[:], AF.Sqrt, bias=eps_t[0:1, :])
            rvf = tp.tile([1, 1], F32, name="fln_rv")
            nc.vector.reciprocal(rvf[:], sdf[:])
            cvf = tp.tile([1, 1], F32, name="fln_cv")
            nc.vector.scalar_tensor_tensor(cvf[:], muf[:], -1.0, rvf[:],
                                           op0=OP.mult, op1=OP.mult)
            rbf = tp.tile([P, 1], F32, name="fln_rb")
            nc.gpsimd.partition_broadcast(rbf[:], rvf[:])
            cbf = tp.tile([P, 1], F32, name="fln_cb")
            nc.gpsimd.partition_broadcast(cbf[:], cvf[:])
            hfin = tp.tile([P, 8], F32, name="hfin")
            for r in range(8):
                t2 = tp.tile([P, 1], F32, name="fln_t2")
                nc.vector.tensor_mul(t2[:], h[:, r, 511:512], rbf[:])
                nc.vector.tensor_add(t2[:], t2[:], cbf[:])
                nc.scalar.activation(hfin[:, r:r + 1], t2[:], AF.Identity,
                                     bias=fln[:, 1, r:r + 1],
                                     scale=fln[:, 0, r:r + 1])
            nc.sync.dma_start(fin_ag_in.ap().rearrange("(r p) -> p r", p=P),
                              hfin[:])
            nc.gpsimd.collective_compute(
                "AllGather", OP.bypass, replica_groups=ALL8,
                ins=[fin_ag_in[:]], outs=[fin_ag_out[:, :]])

            # LM head: my vocab chunk (VC=4000) for all 4 batches
            hall = tp.tile([P, 8, 4], F32R, name="hall")
            for bb in range(4):
                nc.gpsimd.dma_start(
                    hall[:, :, bb],
                    fin_ag_out[2 * bb + 1].rearrange("(r p) -> p r", p=P))
            for nt in range(8):
                n0, n1 = nt * 500, (nt + 1) * 500
                l_ps = ps_mm.tile([P, 512], F32, name="mm")
                for kt in range(8):
                    wl = tp.tile([P, 500], F32R, name="wlm")
                    nc.sync.dma_start(wl[:], wlm_d[kt * P:(kt + 1) * P, n0:n1])
                    nc.tensor.matmul(l_ps[0:4, 0:500], hall[:, kt, :], wl[:],
                                     start=(kt == 0), stop=(kt == 7))
                bl = tp.tile([1, 500], F32, name="blm")
                nc.sync.dma_start(bl[:], blm_d[n0:n1])
                blb = tp.tile([4, 500], F32, name="blb")
                nc.gpsimd.partition_broadcast(blb[:], bl[:])
                lo = tp.tile([4, 512], F32, name="lo")
                nc.vector.tensor_add(lo[:, 0:500], l_ps[0:4, 0:500], blb[:])
                nc.sync.dma_start(logits_d[:, n0:n1], lo[:, 0:500])

    nc.compile()
    return nc


def _pe_table(t, d):
    pos = np.arange(t, dtype=np.float32)[:, None]
    freq = np.exp(-(np.arange(0, d, 2, dtype=np.float32) / d) * np.log(10000.0))
    ang = pos * freq[None, :]
    pe = np.zeros((t, d), dtype=np.float32)
    pe[:, 0::2] = np.sin(ang)
    pe[:, 1::2] = np.cos(ang)
    return pe


def kernel(idx, emb, Wqkv, Wout, ln1_g, ln1_b, ln2_g, ln2_b, Wup, bup,
           Wgate, bgate, Wdown, bdown, fln_g, fln_b, Wlm, blm, _trace=False):
    f32 = lambda x: np.ascontiguousarray(np.asarray(x, dtype=np.float32))
    idx = np.asarray(idx)
    emb, Wqkv, Wup, Wgate, Wdown, Wlm = map(f32, (emb, Wqkv, Wup, Wgate, Wdown, Wlm))
    Wout, blm_f = f32(Wout), f32(blm)

    nl = int(np.asarray(Wqkv).shape[0])
    if ("nc", nl) not in _CACHE:
        _CACHE[("nc", nl)] = build_kernel(nl)
    nc = _CACHE[("nc", nl)]

    pe = _pe_table(T, D)
    in_maps = []
    for core in range(8):
        b, half = core // 2, core % 2
        t0 = half * TL
        in_maps.append({
            "idx": np.ascontiguousarray(idx[b, t0:t0 + TL]).astype(np.uint32),
            "pe": np.ascontiguousarray(pe[t0:t0 + TL]),
            "emb": emb,
            "wqkv": Wqkv,
            "wout": np.ascontiguousarray(Wout[:, t0:t0 + TL, :]),
            "wup": Wup, "wgate": Wgate, "wdown": Wdown,
            "ln1g": f32(ln1_g), "ln1b": f32(ln1_b),
            "ln2g": f32(ln2_g), "ln2b": f32(ln2_b),
            "bup": f32(bup), "bgate": f32(bgate), "bdown": f32(bdown),
            "flng": f32(fln_g), "flnb": f32(fln_b),
            "wlm": np.ascontiguousarray(Wlm[:, core * VC:(core + 1) * VC]),
            "blm": np.ascontiguousarray(blm_f[core * VC:(core + 1) * VC]),
        })
    res = run_bass_kernel_spmd(nc, in_maps, core_ids=list(range(8)),
                               trace=_trace)
    logits = np.zeros((B, 1, V), dtype=np.float32)
    for core in range(8):
        logits[:, 0, core * VC:(core + 1) * VC] = res.results[core]["logits"]
    if _trace:
        return logits, res
    return logits
